# revision 4
# baseline (speedup 1.0000x reference)
"""Maxwell viscoelastic model (linear recurrence scan) on 8 Trainium2 NeuronCores.

Math (per trajectory, T timesteps):
    a_n = 1 - k*dt_n              (k = E/eta = 2)
    h_n = a_n*h_{n-1} - 4*dt_n*eps_n   -> h = -2*gamma
    sigma_n = 2.5*eps_n + h_n

Sharding: batch (4096 trajectories) across 8 cores (512 each); per core
4 tiles of [128 partitions x 4096 timesteps], streamed in CH=4 chunks of
L=1024.  All HBM I/O is fp16 (host casts in/out; tolerance is 2e-2), and
the input is de-interleaved on the host to [B, 2, T] so every on-chip
read is packed (enables the DVE 2-byte 2x perf mode for the sigma add).

Engine split (per chunk q):
  SYNC  loads xt chunks (HWDGE ring)
  PE    a = 1 - 2*dt -> PSUM:  I*ones matmul (start) + (-2I)*dt matmul
        (accumulate).  Identity stationaries + ones tile are tiny host
        inputs.  The tensor engine is otherwise idle, so the affine
        pass is free.
  ACT   e4 = -4*eps -> SBUF fp16,  seps = 2.5*eps -> SBUF fp16, and
        issues output stores on its HWDGE ring.
  POOL  bneg = dt * e4 -> SBUF fp16
  DVE   scan(a[PSUM], bneg[SBUF]) -> h fp16 SBUF   (fp32 internal state)
        sigma = seps + h  (tensor_tensor add, all fp16 SBUF -> 2x mode)

Dependency chain per chunk: load -> ACT(e4) -> POOL(bneg) -> PE(last
matmul waits pool) -> DVE(scan waits only pe_seq, which transitively
covers pool/act/load).  Rings: xt 6 chunks, e4/seps/bneg/h/sig 4, pa 3
(PSUM 6 of 8 banks) so the producers run several chunks ahead and the
DVE never stalls in steady state.
"""

import numpy as np

import concourse.bass as bass
import concourse.mybir as mybir
from concourse.bass_utils import run_bass_kernel_spmd

K = 2.0                      # E/eta
NEG_EK = -4.0                # scan data1 scale: h = -E*gamma = -2*gamma
SIG_EPS = 2.5                # E_infty + E

N_CORES = 8
P = 128
CH = 4                       # time chunks per tile
XS = 6                       # xt ring depth (chunks)
RS = 4                       # e4/seps/bneg/h/sig ring depth
PS = 3                       # pa (PSUM) ring depth
MM = 512                     # max matmul free size (one PSUM bank of f32)


def build_nc(b_shard: int, t_len: int) -> bass.Bass:
    nc = bass.Bass()
    f16 = mybir.dt.float16
    f32 = mybir.dt.float32
    mult = mybir.AluOpType.mult
    add = mybir.AluOpType.add

    x = nc.dram_tensor("x", [b_shard, 2, t_len], f16, kind="ExternalInput")
    wi = nc.dram_tensor("wi", [P, P], f16, kind="ExternalInput")
    wn2 = nc.dram_tensor("wn2", [P, P], f16, kind="ExternalInput")
    ones = nc.dram_tensor("ones", [P, MM], f16, kind="ExternalInput")
    y = nc.dram_tensor("y", [b_shard, t_len], f16, kind="ExternalOutput")

    n_tiles = b_shard // P
    assert n_tiles * P == b_shard and t_len % CH == 0
    L = t_len // CH
    n_mm = (L + MM - 1) // MM
    assert L % n_mm == 0
    Lm = L // n_mm           # matmul piece size (<= MM)
    Q = n_tiles * CH

    xr = x.rearrange("(n p) c t -> n p c t", p=P)   # [n_tiles, 128, 2, T]
    yr = y.rearrange("(n p) t -> n p t", p=P)       # [n_tiles, 128, T]

    def cs(c):
        return slice(c * L, (c + 1) * L)

    with nc.Block(no_gpsimd_drain=True) as block:
        wis = nc.alloc_sbuf_tensor("wis", [P, P], f16)
        wn2s = nc.alloc_sbuf_tensor("wn2s", [P, P], f16)
        oness = nc.alloc_sbuf_tensor("oness", [P, MM], f16)
        xt = [nc.alloc_sbuf_tensor(f"xt{s}", [P, 2, L], f16) for s in range(XS)]
        e4 = [nc.alloc_sbuf_tensor(f"e4_{s}", [P, L], f16) for s in range(RS)]
        seps = [nc.alloc_sbuf_tensor(f"seps{s}", [P, L], f16) for s in range(RS)]
        bneg = [nc.alloc_sbuf_tensor(f"bneg{s}", [P, L], f16) for s in range(RS)]
        hb = [nc.alloc_sbuf_tensor(f"h{s}", [P, L], f16) for s in range(RS)]
        sig = [nc.alloc_sbuf_tensor(f"sig{s}", [P, L], f16) for s in range(RS)]
        pa = [nc.alloc_psum_tensor(f"pa{s}", [P, L], f32) for s in range(PS)]

        sem_in = [nc.alloc_semaphore(f"in{s}") for s in range(XS)]
        sem_out = [nc.alloc_semaphore(f"out{s}") for s in range(RS)]
        sem_const = nc.alloc_semaphore("constload")
        pe_seq = nc.alloc_semaphore("pe_seq")      # +1 per chunk
        act_seq = nc.alloc_semaphore("act_seq")    # +2 per chunk (e4, seps)
        pool_seq = nc.alloc_semaphore("pool_seq")  # +1 per chunk
        dve_seq = nc.alloc_semaphore("dve_seq")    # +2 per chunk (scan, sigma)

        @block.sync
        def _(sync):
            sync.dma_start(wis[:], wi[:]).then_inc(sem_const, 16)
            sync.dma_start(wn2s[:], wn2[:]).then_inc(sem_const, 16)
            sync.dma_start(oness[:], ones[:]).then_inc(sem_const, 16)
            for q in range(Q):
                i, c = divmod(q, CH)
                s = q % XS
                if q >= XS:
                    # xt slot reuse: all readers of chunk q-XS done.
                    sync.wait_ge(pe_seq, q - XS + 1)
                    sync.wait_ge(act_seq, 2 * (q - XS) + 2)
                    sync.wait_ge(pool_seq, q - XS + 1)
                sync.dma_start(xt[s][:, :, :], xr[i][:, :, cs(c)]).then_inc(
                    sem_in[s], 16
                )

        @block.tensor
        def _(tensor):
            tensor.wait_ge(sem_const, 48)
            for q in range(Q):
                s = q % XS
                tensor.wait_ge(sem_in[s], 16 * (q // XS + 1))
                if q >= PS:
                    # pa slot WAR: scan(q-PS) was the reader.
                    tensor.wait_ge(dve_seq, 2 * (q - PS) + 1)
                dt = xt[s][:, 1, :]
                for m in range(n_mm):
                    sl = slice(m * Lm, (m + 1) * Lm)
                    tensor.matmul(
                        pa[q % PS][:, sl], wis[:], oness[:, 0:Lm],
                        start=True, stop=False,
                    )
                    if m == n_mm - 1:
                        # Chain POOL ahead of PE so the DVE scan's single
                        # pe_seq wait transitively covers bneg readiness.
                        tensor.wait_ge(pool_seq, q + 1)
                    mm = tensor.matmul(
                        pa[q % PS][:, sl], wn2s[:], dt[:, sl],
                        start=False, stop=True,
                    )
                mm.then_inc(pe_seq, 1)

        @block.scalar
        def _(scalar):
            def store(k):
                i, c = divmod(k, CH)
                scalar.wait_ge(dve_seq, 2 * k + 2)   # sigma(k) complete
                scalar.dma_start(yr[i][:, cs(c)], sig[k % RS][:]).then_inc(
                    sem_out[k % RS], 16
                )

            for q in range(Q):
                s = q % XS
                scalar.wait_ge(sem_in[s], 16 * (q // XS + 1))
                if q >= RS:
                    # e4 slot WAR: bneg(q-RS) was the reader.
                    scalar.wait_ge(pool_seq, q - RS + 1)
                scalar.activation(
                    e4[q % RS][:], xt[s][:, 0, :],
                    mybir.ActivationFunctionType.Copy, bias=0.0, scale=NEG_EK,
                ).then_inc(act_seq, 1)
                if q >= RS:
                    # seps slot WAR: sigma(q-RS) was the reader.
                    scalar.wait_ge(dve_seq, 2 * (q - RS) + 2)
                scalar.activation(
                    seps[q % RS][:], xt[s][:, 0, :],
                    mybir.ActivationFunctionType.Copy, bias=0.0, scale=SIG_EPS,
                ).then_inc(act_seq, 1)
                if q >= 1:
                    store(q - 1)
            store(Q - 1)
            for s in range(RS):
                rounds = Q // RS + (1 if s < Q % RS else 0)
                scalar.wait_ge(sem_out[s], 16 * rounds)

        @block.gpsimd
        def _(gpsimd):
            for q in range(Q):
                s = q % XS
                gpsimd.wait_ge(sem_in[s], 16 * (q // XS + 1))
                gpsimd.wait_ge(act_seq, 2 * q + 1)   # e4(q) ready
                if q >= RS:
                    # bneg slot WAR: scan(q-RS) was the reader.
                    gpsimd.wait_ge(dve_seq, 2 * (q - RS) + 1)
                gpsimd.tensor_tensor(
                    bneg[q % RS][:], xt[s][:, 1, :], e4[q % RS][:], mult
                ).then_inc(pool_seq, 1)

        @block.vector
        def _(vector):
            for q in range(Q):
                c = q % CH
                vector.wait_ge(pe_seq, q + 1)  # covers pool/act-e4/load too
                if c != 0:
                    # init reads h(q-1) last element: same-engine RAW needs
                    # an explicit wait (the engine pipelines instructions).
                    vector.wait_ge(dve_seq, 2 * q - 1)
                init = 0.0 if c == 0 else hb[(q - 1) % RS][:, L - 1:L]
                vector.tensor_tensor_scan(
                    hb[q % RS][:], pa[q % PS][:], bneg[q % RS][:], init,
                    mult, add,
                ).then_inc(dve_seq, 1)
                vector.wait_ge(dve_seq, 2 * q + 1)   # scan(q) complete (RAW h)
                vector.wait_ge(act_seq, 2 * q + 2)   # seps(q) ready
                if q >= RS:
                    # sig slot WAR: store(q-RS) complete.
                    vector.wait_ge(sem_out[q % RS], 16 * (q // RS))
                vector.tensor_tensor(
                    sig[q % RS][:], seps[q % RS][:], hb[q % RS][:], add
                ).then_inc(dve_seq, 1)

    return nc


_NC_CACHE: dict = {}


def _get_nc(b_shard: int, t_len: int) -> bass.Bass:
    key = (b_shard, t_len)
    if key not in _NC_CACHE:
        _NC_CACHE[key] = build_nc(b_shard, t_len)
    return _NC_CACHE[key]


def make_inputs(x: np.ndarray):
    """Shard + convert the full f32 input for the 8 cores."""
    b, t_len, c = x.shape
    assert c == 2 and b % N_CORES == 0
    b_shard = b // N_CORES
    # [B, T, 2] -> [cores, b_shard, 2, T] fp16 (de-interleaved, packed)
    xs = (
        np.asarray(x, dtype=np.float32)
        .reshape(N_CORES, b_shard, t_len, 2)
        .transpose(0, 1, 3, 2)
        .astype(np.float16)
    )
    xs = np.ascontiguousarray(xs)
    eye = np.eye(P, dtype=np.float16)
    wn2 = (-K * np.eye(P)).astype(np.float16)
    onesv = np.ones((P, MM), dtype=np.float16)
    return [
        {"x": xs[i], "wi": eye, "wn2": wn2, "ones": onesv}
        for i in range(N_CORES)
    ]


def run(x: np.ndarray, trace: bool = False):
    b, t_len, _ = x.shape
    in_maps = make_inputs(x)
    res = run_bass_kernel_spmd(
        _get_nc(b // N_CORES, t_len), in_maps,
        core_ids=list(range(N_CORES)), trace=trace,
    )
    out = np.concatenate([r["y"] for r in res.results], axis=0)
    return out.astype(np.float32).reshape(b, t_len, 1), res


def kernel(x: np.ndarray) -> np.ndarray:
    out, _ = run(x, trace=False)
    return out
